# revision 18
# baseline (speedup 1.0000x reference)
"""Trainium2 Bass kernel for DiffusionTimePredictor.

Computes, for each (b,h) attention head:
  scores = Q @ K^T / sqrt(64)            (2048 x 2048)
  mean_sim = mean(scores)
  max_sim  = mean(rowmax(scores))
  entropy_proxy = mean(var_unbiased(softmax(scores), axis=-1))
  -> 3 features -> tiny MLP -> t = exp(clip(logit))

Sharding: 32 (b,h) heads data-parallel over 8 NeuronCores (4 heads/core).
MLP params replicated; each core computes its own 4 outputs on device.

Key implementation notes:
- scores are built with bf16 matmuls (fp32 accumulate in PSUM).
- mean_sim uses sum(scores) == sum_d (sum_q Q[q,d]) * (sum_k K[k,d]) / 8,
  so it never touches the score matrix.
- Row sums of exp(s) (softmax denominator Z) come for free from the scalar
  engine's activation accumulator during the exp pass.
- sum(exp(s)^2) per row is needed for var(softmax): split between a second
  scalar-engine exp(2s) pass and the vector engine's fused
  tensor_tensor_reduce(e*e), to balance the two engines.
- Row max via a bf16 tensor_tensor(max) cascade (2x DVE mode) + final
  reduce_max; rowmax(s) = 8*ln(rowmax(e)).
- Q^T/K^T layouts are produced by the DMA transpose xbar from a bf16 DRAM
  scratch written by a casting (gpsimd) DMA. The (1024,128) view of the
  (2048,64) matrix transposes to an interleaved layout: partitions 0:64 hold
  even seq rows, 64:128 odd seq rows.  All row/column permutations of the
  score matrix leave every reduced statistic unchanged, so the permuted
  order is used directly.  A partition-swapped copy of Q^T lets the matmuls
  run as two 64-contraction streams in the top and bottom halves of the PE
  array.
"""

import math
import os

import numpy as np

import concourse.bacc as bacc
import concourse.bass as bass
import concourse.mybir as mybir
import concourse.tile as tile
from concourse.bass_utils import run_bass_kernel_spmd

F32 = mybir.dt.float32
BF16 = mybir.dt.bfloat16
AF = mybir.ActivationFunctionType
ALU = mybir.AluOpType
AX = mybir.AxisListType

B, H, SQ, SK, D = 4, 8, 2048, 2048, 64
NCORES = 8
BH = B * H
BHC = BH // NCORES  # heads per core = 4
NQT = SQ // 128  # q tiles per head = 16
LOG_T_MIN = math.log(0.1)
LOG_T_MAX = math.log(10.0)

# which q-tiles route their sum(e^2) to the scalar engine (exp(2s) pass);
# the rest use the vector engine's fused square+accumulate.  7/16 on ACT.
ACT_S2_TILES = {1, 3, 5, 7, 9, 11, 13}
# levels of pairwise bf16 max cascade before the final reduce_max (0 = none)
CASCADE_LEVELS = 4

_CACHE = {}
LAST_RESULTS = None


def _build(repeat=None):
    """Build the per-core bass module.  With repeat=K, the whole computation
    is wrapped in a Tile For_i loop executing K times — used only for device
    timing (marginal time per iteration removes host dispatch overhead)."""
    from contextlib import nullcontext

    nc = bacc.Bacc("TRN2", target_bir_lowering=False, debug=False)

    q_d = nc.dram_tensor("q", [BHC, SQ, D], F32, kind="ExternalInput")
    k_d = nc.dram_tensor("k", [BHC, SK, D], F32, kind="ExternalInput")
    w1_d = nc.dram_tensor("W1", [3, 64], F32, kind="ExternalInput")
    b1_d = nc.dram_tensor("b1", [64], F32, kind="ExternalInput")
    w2_d = nc.dram_tensor("W2", [64, 64], F32, kind="ExternalInput")
    b2_d = nc.dram_tensor("b2", [64], F32, kind="ExternalInput")
    w3_d = nc.dram_tensor("W3", [64, 1], F32, kind="ExternalInput")
    b3_d = nc.dram_tensor("b3", [1], F32, kind="ExternalInput")
    out_d = nc.dram_tensor("out", [BHC], F32, kind="ExternalOutput")

    scq_d = nc.dram_tensor("scq", [BHC, SQ, D], BF16)
    sck_d = nc.dram_tensor("sck", [BHC, SK, D], BF16)

    # persistent SBUF state
    zbias = nc.alloc_sbuf_tensor("zbias", [128, 1], F32).ap()
    ones = nc.alloc_sbuf_tensor("onesv", [128, 1], F32).ap()
    Zt = nc.alloc_sbuf_tensor("Zt", [128, BHC * NQT], F32).ap()    # row sum e
    S2a = nc.alloc_sbuf_tensor("S2a", [128, BHC * NQT], F32).ap()  # row sum e^2 (ACT)
    S2d = nc.alloc_sbuf_tensor("S2d", [128, BHC * NQT], F32).ap()  # row sum e^2 (DVE)
    Mt = nc.alloc_sbuf_tensor("Mt", [128, BHC * NQT], F32).ap()    # row max e
    qsB = nc.alloc_sbuf_tensor("qsB", [128, BHC], F32).ap()
    kk = nc.alloc_sbuf_tensor("kk", [128, 2 * BHC], F32).ap()      # [ks, ksw] per head
    fin = nc.alloc_sbuf_tensor("fin", [128, 32 * BHC], F32).ap()   # [sp2(16) lnM(16)] per head
    S2s = nc.alloc_sbuf_tensor("S2s", [128, BHC * NQT], F32).ap()  # S2a+S2d
    rcp = nc.alloc_sbuf_tensor("rcp", [128, NQT], F32).ap()
    t1s = nc.alloc_sbuf_tensor("t1s", [128, NQT], F32).ap()
    Fs = nc.alloc_sbuf_tensor("Fs", [1, 32 * BHC], F32).ap()
    Gs = nc.alloc_sbuf_tensor("Gs", [1, 2 * BHC], F32).ap()
    Ss = nc.alloc_sbuf_tensor("Ss", [1, 2 * BHC], F32).ap()
    G2 = nc.alloc_sbuf_tensor("G2", [1, BHC], F32).ap()
    stg = nc.alloc_sbuf_tensor("stg", [1, 3 * BHC], F32).ap()
    feat = nc.alloc_sbuf_tensor("feat", [3, BHC], F32).ap()
    w1s = nc.alloc_sbuf_tensor("w1s", [3, 64], F32).ap()
    b1s = nc.alloc_sbuf_tensor("b1s", [64, 1], F32).ap()
    w2s = nc.alloc_sbuf_tensor("w2s", [64, 64], F32).ap()
    b2s = nc.alloc_sbuf_tensor("b2s", [64, 1], F32).ap()
    w3s = nc.alloc_sbuf_tensor("w3s", [64, 1], F32).ap()
    b3s = nc.alloc_sbuf_tensor("b3s", [1, 1], F32).ap()
    h1s = nc.alloc_sbuf_tensor("h1s", [64, BHC], F32).ap()
    h2s = nc.alloc_sbuf_tensor("h2s", [64, BHC], F32).ap()
    tlog = nc.alloc_sbuf_tensor("tlog", [1, BHC], F32).ap()
    tclip = nc.alloc_sbuf_tensor("tclip", [1, BHC], F32).ap()
    tout = nc.alloc_sbuf_tensor("tout", [1, BHC], F32).ap()

    with tile.TileContext(nc) as tc:
      with tc.For_i(0, repeat, 1) if repeat else nullcontext():
        with (
            tc.tile_pool(name="tr", bufs=2) as trp,
            tc.tile_pool(name="ep", bufs=3) as epp,
            tc.tile_pool(name="scr", bufs=2) as scrp,
            tc.tile_pool(name="casc", bufs=2) as cascp,
            tc.tile_pool(name="psB", bufs=2, space=bass.MemorySpace.PSUM) as psB,
        ):
            nc.vector.memset(zbias, 0.0)
            nc.vector.memset(ones, 1.0)
            nc.vector.memset(S2a, 0.0)
            nc.vector.memset(S2d, 0.0)

            # MLP params
            nc.sync.dma_start(out=w1s, in_=w1_d[:])
            nc.sync.dma_start(out=b1s, in_=b1_d[:].rearrange("(a b) -> a b", b=1))
            nc.sync.dma_start(out=w2s, in_=w2_d[:])
            nc.sync.dma_start(out=b2s, in_=b2_d[:].rearrange("(a b) -> a b", b=1))
            nc.sync.dma_start(out=w3s, in_=w3_d[:])
            nc.sync.dma_start(out=b3s, in_=b3_d[:].rearrange("(a b) -> a b", b=1))

            for bh in range(BHC):
                # ---- preprocessing: cast to bf16 in DRAM, transpose-load ----
                nc.gpsimd.dma_start(out=scq_d[bh][:], in_=q_d[bh][:])
                nc.gpsimd.dma_start(out=sck_d[bh][:], in_=k_d[bh][:])
                QT = trp.tile([128, SQ // 2], BF16, tag="QT")
                KT = trp.tile([128, SK // 2], BF16, tag="KT")
                QTs = trp.tile([128, SQ // 2], BF16, tag="QTs")
                nc.sync.dma_start_transpose(
                    QT[:], scq_d[bh][:].rearrange("(a b) d -> a (b d)", b=2)
                )
                nc.sync.dma_start_transpose(
                    KT[:], sck_d[bh][:].rearrange("(a b) d -> a (b d)", b=2)
                )
                # partition-swapped copy of Q^T
                nc.sync.dma_start(out=QTs[64:128, :], in_=QT[0:64, :])
                nc.sync.dma_start(out=QTs[0:64, :], in_=QT[64:128, :])

                # column sums of Q and K (for mean_sim)
                nc.vector.reduce_sum(qsB[:, bh : bh + 1], QT[:], axis=AX.X)
                nc.vector.reduce_sum(kk[:, 2 * bh : 2 * bh + 1], KT[:], axis=AX.X)

                for t in range(NQT):
                    even = t < NQT // 2
                    ct = 128 * (t % (NQT // 2))
                    lo_stat = (QT if even else QTs)[0:64, ct : ct + 128]
                    hi_stat = (QTs if even else QT)[64:128, ct : ct + 128]
                    gt = bh * NQT + t

                    P = psB.tile([128, 2048], F32, tag="P")
                    nc.tensor.matmul(
                        P[:, 0:512], lo_stat, KT[0:64, 0:512], start=True, stop=True
                    )
                    nc.tensor.matmul(
                        P[:, 512:1024], lo_stat, KT[0:64, 512:1024],
                        start=True, stop=True,
                    )
                    nc.tensor.matmul(
                        P[:, 1024:1536], hi_stat, KT[64:128, 0:512],
                        start=True, stop=True, tile_position=(64, 0),
                    )
                    nc.tensor.matmul(
                        P[:, 1536:2048], hi_stat, KT[64:128, 512:1024],
                        start=True, stop=True, tile_position=(64, 0),
                    )

                    e = epp.tile([128, 2048], BF16, tag="e")
                    nc.scalar.activation(
                        e[:], P[:], AF.Exp, bias=zbias, scale=0.125,
                        accum_out=Zt[:, gt : gt + 1],
                    )
                    scr = scrp.tile([128, 2048], BF16, tag="scr")
                    if t in ACT_S2_TILES:
                        # exp(2s) straight from the PSUM scores — same Exp
                        # table as the main pass (Square would thrash the
                        # ACT table set: measured +50us).
                        nc.scalar.activation(
                            scr[:], P[:], AF.Exp, bias=zbias, scale=0.25,
                            accum_out=S2a[:, gt : gt + 1],
                        )
                    else:
                        nc.vector.scalar_tensor_tensor(
                            scr[:], in0=e[:], scalar=1.0, in1=e[:],
                            op0=ALU.mult, op1=ALU.mult,
                            accum_out=S2d[:, gt : gt + 1],
                        )
                    # row max cascade (bf16, 2x DVE mode) + final reduce
                    cur = e
                    width = 2048
                    for lv in range(CASCADE_LEVELS):
                        width //= 2
                        nxt = cascp.tile([128, width], BF16, tag=f"m{lv}")
                        nc.vector.tensor_max(
                            nxt[:], cur[:, 0:width], cur[:, width : 2 * width]
                        )
                        cur = nxt
                    nc.vector.reduce_max(Mt[:, gt : gt + 1], cur[:], axis=AX.X)

        # ---------------- tail: features + MLP ----------------
        with tc.tile_pool(name="psT", bufs=1, space=bass.MemorySpace.PSUM) as psT:
            # swapped-half copies of K column sums (for the cross terms of
            # sum_d Qsum*Ksum with the even/odd split layout)
            with nc.allow_non_contiguous_dma(reason="tiny 4-elem column swap"):
                nc.sync.dma_start(
                    out=kk[0:64, 1 : 2 * BHC : 2], in_=kk[64:128, 0 : 2 * BHC : 2]
                )
                nc.sync.dma_start(
                    out=kk[64:128, 1 : 2 * BHC : 2], in_=kk[0:64, 0 : 2 * BHC : 2]
                )

            nc.vector.tensor_add(S2s, S2a, S2d)
            for bh in range(BHC):
                c0 = bh * NQT
                nc.vector.reciprocal(rcp, Zt[:, c0 : c0 + NQT])
                nc.vector.tensor_mul(t1s, S2s[:, c0 : c0 + NQT], rcp)
                nc.vector.tensor_mul(fin[:, bh * 32 : bh * 32 + 16], t1s, rcp)
                nc.scalar.activation(
                    fin[:, bh * 32 + 16 : bh * 32 + 32],
                    Mt[:, c0 : c0 + NQT], AF.Ln, bias=zbias, scale=1.0,
                )

            PF = psT.tile([1, 32 * BHC], F32, tag="PF")
            nc.tensor.matmul(PF[:], ones, fin, start=True, stop=True)
            PD = psT.tile([1, 2 * BHC], F32, tag="PD")
            for bh in range(BHC):
                nc.tensor.matmul(
                    PD[:, 2 * bh : 2 * bh + 2],
                    qsB[:, bh : bh + 1],
                    kk[:, 2 * bh : 2 * bh + 2],
                    start=True, stop=True,
                )
            nc.vector.tensor_copy(Fs, PF[:])
            nc.vector.tensor_copy(Gs, PD[:])

            nc.vector.reduce_sum(
                Ss, Fs.rearrange("p (g c) -> p g c", c=16), axis=AX.X
            )
            nc.vector.reduce_sum(
                G2, Gs.rearrange("p (g c) -> p g c", c=2), axis=AX.X
            )
            # features:
            # mean_sim = dot/(8*SQ*SK)
            nc.vector.tensor_scalar_mul(
                stg[:, 0:BHC], G2, 1.0 / (8.0 * SQ * SK)
            )
            # max_sim = mean(ln rowmax e)  (ln(max e) is already the score max)
            nc.vector.tensor_scalar_mul(
                stg[:, BHC : 2 * BHC], Ss[:, 1 : 2 * BHC : 2], 1.0 / SQ
            )
            # entropy = (sum(sp2) - 1) / (SK*(SK-1))
            cent = 1.0 / (float(SK) * (SK - 1.0))
            nc.vector.tensor_scalar(
                stg[:, 2 * BHC : 3 * BHC], Ss[:, 0 : 2 * BHC : 2],
                scalar1=cent, scalar2=-cent, op0=ALU.mult, op1=ALU.add,
            )
            # scatter rows to partitions 0..2
            nc.sync.dma_start(out=feat[0:1, :], in_=stg[:, 0:BHC])
            nc.sync.dma_start(out=feat[1:2, :], in_=stg[:, BHC : 2 * BHC])
            nc.sync.dma_start(out=feat[2:3, :], in_=stg[:, 2 * BHC : 3 * BHC])

            # MLP
            PM1 = psT.tile([64, BHC], F32, tag="PM1")
            nc.tensor.matmul(PM1[:], w1s, feat, start=True, stop=True)
            nc.scalar.activation(h1s, PM1[:], AF.Gelu, bias=b1s, scale=1.0)
            PM2 = psT.tile([64, BHC], F32, tag="PM2")
            nc.tensor.matmul(PM2[:], w2s, h1s, start=True, stop=True)
            nc.scalar.activation(h2s, PM2[:], AF.Gelu, bias=b2s, scale=1.0)
            PM3 = psT.tile([1, BHC], F32, tag="PM3")
            nc.tensor.matmul(PM3[:], w3s, h2s, start=True, stop=True)
            nc.scalar.activation(tlog, PM3[:], AF.Identity, bias=b3s, scale=1.0)
            nc.vector.tensor_scalar(
                tclip, tlog, scalar1=LOG_T_MIN, scalar2=LOG_T_MAX,
                op0=ALU.max, op1=ALU.min,
            )
            nc.scalar.activation(tout, tclip, AF.Exp, bias=zbias[0:1, :], scale=1.0)
            nc.sync.dma_start(
                out=out_d[:].rearrange("(a b) -> a b", a=1), in_=tout
            )

    nc.compile()
    return nc


class _Runner:
    """Caches the jitted shard_map executable for the compiled bass module so
    repeated invocations (timing loops) don't re-trace/re-compile.  Mirrors
    concourse.bass2jax.run_bass_via_pjrt's multi-core path."""

    def __init__(self, nc, n_cores):
        import jax
        from jax.sharding import Mesh, PartitionSpec
        from jax.experimental.shard_map import shard_map
        from concourse import bass2jax as b2j

        b2j.install_neuronx_cc_hook()
        self.nc = nc
        self.n_cores = n_cores
        in_names, out_names, out_avals, zero_outs = [], [], [], []
        partition_name = (
            nc.partition_id_tensor.name if nc.partition_id_tensor else None
        )
        for alloc in nc.m.functions[0].allocations:
            if not isinstance(alloc, mybir.MemoryLocationSet):
                continue
            name = alloc.memorylocations[0].name
            if alloc.kind == "ExternalInput":
                if name != partition_name:
                    in_names.append(name)
            elif alloc.kind == "ExternalOutput":
                out_names.append(name)
                shape = tuple(alloc.tensor_shape)
                dtype = mybir.dt.np(alloc.dtype)
                out_avals.append(jax.core.ShapedArray(shape, dtype))
                zero_outs.append(np.zeros(shape, dtype))
        n_params = len(in_names)
        n_outs = len(out_avals)
        in_names = in_names + out_names
        if partition_name is not None:
            in_names.append(partition_name)
        self.in_names = in_names
        self.out_names = out_names
        self.out_avals = out_avals
        self.n_params = n_params
        self.zero_outs = zero_outs
        donate = tuple(range(n_params, n_params + n_outs))

        def _body(*args):
            operands = list(args)
            if partition_name is not None:
                operands.append(b2j.partition_id_tensor())
            outs = b2j._bass_exec_p.bind(
                *operands,
                out_avals=tuple(out_avals),
                in_names=tuple(in_names),
                out_names=tuple(out_names),
                lowering_input_output_aliases=(),
                sim_require_finite=True,
                sim_require_nnan=True,
                nc=nc,
            )
            return tuple(outs)

        devices = jax.devices()[:n_cores]
        self.mesh = Mesh(np.asarray(devices), ("core",))
        in_specs = (PartitionSpec("core"),) * (n_params + n_outs)
        out_specs = (PartitionSpec("core"),) * n_outs
        self._fn = jax.jit(
            shard_map(
                _body,
                mesh=self.mesh,
                in_specs=in_specs,
                out_specs=out_specs,
                check_rep=False,
            ),
            donate_argnums=donate,
            keep_unused=True,
        )
        self._jax = jax

    def concat_inputs(self, in_maps):
        per_core = [
            [np.asarray(m[name]) for name in self.in_names[: self.n_params]]
            for m in in_maps
        ]
        return [
            np.concatenate([per_core[c][i] for c in range(self.n_cores)], axis=0)
            for i in range(self.n_params)
        ]

    def _zeros(self):
        return [
            np.zeros((self.n_cores * z.shape[0], *z.shape[1:]), z.dtype)
            for z in self.zero_outs
        ]

    def run(self, concat_in):
        out_arrs = self._fn(*concat_in, *self._zeros())
        out_arrs = [np.asarray(o) for o in out_arrs]
        return [
            {
                name: out_arrs[i].reshape(self.n_cores, *self.out_avals[i].shape)[c]
                for i, name in enumerate(self.out_names)
            }
            for c in range(self.n_cores)
        ]

    def time(self, concat_in, iters=30):
        import time as _time

        dev_in = [self._jax.device_put(x) for x in concat_in]
        # warmup (also triggers compile)
        self._fn(*dev_in, *self._zeros())[0].block_until_ready()
        times = []
        for _ in range(iters):
            zs = self._zeros()
            t0 = _time.perf_counter()
            out = self._fn(*dev_in, *zs)
            out[0].block_until_ready()
            times.append(_time.perf_counter() - t0)
        return times

    def _make_chain(self, n):
        import jax
        from jax.sharding import PartitionSpec
        from jax.experimental.shard_map import shard_map
        from concourse import bass2jax as b2j

        nc = self.nc
        out_avals = tuple(self.out_avals)
        in_names = tuple(self.in_names)
        out_names = tuple(self.out_names)
        n_params = self.n_params
        n_outs = len(out_names)

        def _body_n(*args):
            # n independent effect-ordered executions on the same operands —
            # operands stay direct parameters (the neuronx hook requires it).
            for _ in range(n):
                outs = b2j._bass_exec_p.bind(
                    *args,
                    out_avals=out_avals,
                    in_names=in_names,
                    out_names=out_names,
                    lowering_input_output_aliases=(),
                    sim_require_finite=True,
                    sim_require_nnan=True,
                    nc=nc,
                )
            return tuple(outs)

        in_specs = (PartitionSpec("core"),) * (n_params + n_outs)
        out_specs = (PartitionSpec("core"),) * n_outs
        donate = tuple(range(n_params, n_params + n_outs))
        return jax.jit(
            shard_map(
                _body_n,
                mesh=self.mesh,
                in_specs=in_specs,
                out_specs=out_specs,
                check_rep=False,
            ),
            donate_argnums=donate,
            keep_unused=True,
        )

    def time_chained(self, concat_in, n_small=2, n_big=34, reps=6):
        """Amortized per-execution device time: chain n kernel executions in
        one jitted program; marginal time removes the dispatch overhead."""
        import time as _time

        dev_in = [self._jax.device_put(x) for x in concat_in]
        fn_s = self._make_chain(n_small)
        fn_b = self._make_chain(n_big)

        def run(fn):
            out = fn(*dev_in, *self._zeros())
            out[0].block_until_ready()

        run(fn_s)
        run(fn_b)  # warmup/compile
        ts, tb = [], []
        for _ in range(reps):
            t0 = _time.perf_counter()
            run(fn_s)
            ts.append(_time.perf_counter() - t0)
            t0 = _time.perf_counter()
            run(fn_b)
            tb.append(_time.perf_counter() - t0)
        ts_m = sorted(ts)[len(ts) // 2]
        tb_m = sorted(tb)[len(tb) // 2]
        per_exec = (tb_m - ts_m) / (n_big - n_small)
        return per_exec, ts_m, tb_m


def kernel(**inputs):
    global LAST_RESULTS
    if "nc" not in _CACHE:
        _CACHE["nc"] = _build()
        _CACHE["runner"] = _Runner(_CACHE["nc"], NCORES)
    nc = _CACHE["nc"]

    q = np.ascontiguousarray(np.asarray(inputs["query"], dtype=np.float32)).reshape(
        BH, SQ, D
    )
    k = np.ascontiguousarray(np.asarray(inputs["key"], dtype=np.float32)).reshape(
        BH, SK, D
    )
    shared = {
        "W1": np.ascontiguousarray(np.asarray(inputs["W1"], dtype=np.float32)),
        "b1": np.ascontiguousarray(np.asarray(inputs["b1"], dtype=np.float32)),
        "W2": np.ascontiguousarray(np.asarray(inputs["W2"], dtype=np.float32)),
        "b2": np.ascontiguousarray(np.asarray(inputs["b2"], dtype=np.float32)),
        "W3": np.ascontiguousarray(np.asarray(inputs["W3"], dtype=np.float32)),
        "b3": np.ascontiguousarray(np.asarray(inputs["b3"], dtype=np.float32)),
    }
    in_maps = []
    for c in range(NCORES):
        m = dict(shared)
        m["q"] = np.ascontiguousarray(q[c * BHC : (c + 1) * BHC])
        m["k"] = np.ascontiguousarray(k[c * BHC : (c + 1) * BHC])
        in_maps.append(m)

    runner = _CACHE["runner"]
    concat_in = runner.concat_inputs(in_maps)
    results = runner.run(concat_in)
    LAST_RESULTS = results
    t = np.concatenate([results[i]["out"] for i in range(NCORES)])
    return t.reshape(B, H, 1, 1).astype(np.float32)


def _make_in_maps(inputs):
    q = np.asarray(inputs["query"], dtype=np.float32).reshape(BH, SQ, D)
    k = np.asarray(inputs["key"], dtype=np.float32).reshape(BH, SK, D)
    shared = {
        n: np.ascontiguousarray(np.asarray(inputs[n], dtype=np.float32))
        for n in ("W1", "b1", "W2", "b2", "W3", "b3")
    }
    in_maps = []
    for c in range(NCORES):
        m = dict(shared)
        m["q"] = np.ascontiguousarray(q[c * BHC : (c + 1) * BHC])
        m["k"] = np.ascontiguousarray(k[c * BHC : (c + 1) * BHC])
        in_maps.append(m)
    return in_maps


def time_kernel(iters=30, **inputs):
    """Returns list of per-call wall times (s) for the cached executable."""
    kernel(**inputs)  # ensure built + correct path warm
    runner = _CACHE["runner"]
    return runner.time(runner.concat_inputs(_make_in_maps(inputs)), iters=iters)


def time_kernel_device(k_small=1, k_big=33, pipeline=24, reps=3, **inputs):
    """True device time per kernel execution.

    Builds two For_i-wrapped modules that run the whole computation K times
    on device; the marginal wall time per extra iteration, measured with a
    pipelined stream of dispatches, is the device execution time (host/axon
    dispatch overhead and I/O transfer cancel in the difference)."""
    import time as _time

    for key, K in (("rs", k_small), ("rb", k_big)):
        if key not in _CACHE:
            _CACHE[key] = _Runner(_build(repeat=K), NCORES)
    rs, rb = _CACHE["rs"], _CACHE["rb"]
    ci = rs.concat_inputs(_make_in_maps(inputs))
    dev_in = [rs._jax.device_put(x) for x in ci]

    def run_stream(r, n):
        outs = [r._fn(*dev_in, *r._zeros()) for _ in range(n)]
        outs[-1][0].block_until_ready()

    run_stream(rs, 2)
    run_stream(rb, 2)  # warm/compile
    t_s, t_b = [], []
    for _ in range(reps):
        t0 = _time.perf_counter()
        run_stream(rs, pipeline)
        t_s.append((_time.perf_counter() - t0) / pipeline)
        t0 = _time.perf_counter()
        run_stream(rb, pipeline)
        t_b.append((_time.perf_counter() - t0) / pipeline)
    ts_m = sorted(t_s)[len(t_s) // 2]
    tb_m = sorted(t_b)[len(t_b) // 2]
    per_exec = (tb_m - ts_m) / (k_big - k_small)
    return per_exec, ts_m, tb_m


def predict_timeline(trace_path=None):
    """Cost-model predicted kernel time in ns (single core), optional perfetto."""
    from concourse.timeline_sim import TimelineSim

    if "nc" not in _CACHE:
        _CACHE["nc"] = _build()
    ts = TimelineSim(_CACHE["nc"], trace=trace_path is not None)
    total = ts.simulate()
    if trace_path is not None and ts.perfetto is not None:
        ts.perfetto.save(trace_path)
    return total


if __name__ == "__main__":
    rng = np.random.default_rng(0)
    ins = {
        "query": rng.standard_normal((B, H, SQ, D), dtype=np.float32),
        "key": rng.standard_normal((B, H, SK, D), dtype=np.float32),
        "W1": rng.standard_normal((3, 64), dtype=np.float32) * 0.1,
        "b1": np.zeros(64, np.float32),
        "W2": rng.standard_normal((64, 64), dtype=np.float32) * 0.1,
        "b2": np.zeros(64, np.float32),
        "W3": np.zeros((64, 1), np.float32),
        "b3": np.zeros(1, np.float32),
    }
    print(kernel(**ins))
